# revision 2
# baseline (speedup 1.0000x reference)
import numpy as np
import jax
import jax.numpy as jnp

# Problem constants (nn_AdaTTSp): hardcoded per harness rules.
L, T, E, D, H = 2, 8, 2, 128, 128
NE = T * E  # 16
M = 8  # number of NeuronCores; data-parallel over batch

_BF = jnp.bfloat16
_F32 = jnp.float32


def _forward(x, w1, b1, w2, b2, gate_w, gate_b, sewf):
    # x: [b, T, D] local shard. Weights pre-cast to bf16 on host; biases f32.
    # sewf: [L, T, NE] — self-expert residual pre-scattered into gate space.
    for l in range(L):
        xb = x.astype(_BF)
        # Expert MLP: w1[l] reshaped [T, E, D, H] so no repeat() is needed.
        h = jax.nn.relu(
            jnp.einsum('btd,tedh->bteh', xb, w1[l],
                       preferred_element_type=_F32) + b1[l])
        eo = jax.nn.relu(
            jnp.einsum('bteh,teho->bteo', h.astype(_BF), w2[l],
                       preferred_element_type=_F32) + b2[l])  # [b,T,E,H]
        eo = eo.reshape(eo.shape[0], NE, H)
        # Gating over all NE experts per task; fold self-expert residual in.
        logits = jnp.einsum('btd,tde->bte', xb, gate_w[l],
                            preferred_element_type=_F32) + gate_b[l]
        coef = jax.nn.softmax(logits, axis=-1) + sewf[l]  # [b, T, NE]
        x = jnp.einsum('bte,beh->bth', coef.astype(_BF), eo.astype(_BF),
                       preferred_element_type=_F32)
    return x


_pfwd = jax.pmap(_forward, axis_name='x',
                 in_axes=(0, None, None, None, None, None, None, None))


def _prep(w1, b1, w2, b2, gate_w, gate_b, sew):
    # Host-side weight prep (tiny tensors): layouts + bf16 cast + sew scatter.
    w1r = np.asarray(w1, np.float32).reshape(L, T, E, D, H)
    b1r = np.asarray(b1, np.float32).reshape(L, T, E, H)
    w2r = np.asarray(w2, np.float32).reshape(L, T, E, H, H)
    b2r = np.asarray(b2, np.float32).reshape(L, T, E, H)
    sewf = np.zeros((L, T, NE), np.float32)
    for t in range(T):
        for e in range(E):
            sewf[:, t, t * E + e] = np.asarray(sew)[:, t, e]
    bf = jnp.bfloat16
    return (jnp.asarray(w1r, bf), jnp.asarray(b1r), jnp.asarray(w2r, bf),
            jnp.asarray(b2r), jnp.asarray(np.asarray(gate_w, np.float32), bf),
            jnp.asarray(np.asarray(gate_b, np.float32)), jnp.asarray(sewf))


def kernel(inputs, w1, b1, w2, b2, gate_w, gate_b, sew):
    B = inputs.shape[0]
    xs = np.asarray(inputs).reshape(M, B // M, T, D)
    wargs = _prep(w1, b1, w2, b2, gate_w, gate_b, sew)
    out = _pfwd(xs, *wargs)
    return np.asarray(out).reshape(B, T, H).astype(np.float32)
